# revision 2
# baseline (speedup 1.0000x reference)
"""Trainium2 Bass kernel for nn_LoopedMLP (moe_routing).

Reference semantics (B=8, T=1024, C=1024, ITER=4096, FULL=12288, R=0.7):
a 3-step scan; each step computes
    y = relu((x @ Wm^T) * active_mask) @ Wp^T
then a control net on mean-pooled y picks a new top-4096 column mask, and a
batch-mean "novelty" below R freezes the state for remaining steps.

Because relu((x@Wm^T)*mask) @ Wp^T only touches the masked 4096 columns, each
step is exactly a dense per-sample MLP over the *gathered* active columns:
    y = relu(x @ A^T) @ B,   A = Wm[idx], B = Wp[:, idx]^T,  |idx| = 4096.

Strategy: data-parallel over B (1 sample per NeuronCore, 8 cores). The device
kernel computes the gathered MLP; all routing logic (control net, top-k,
novelty, done) runs on host between launches. With random weights the step-1
novelty is ~2/3 < R, so in practice exactly one device launch happens.

Device compute per core: two back-to-back matmul chains in fp16 (stationary
weights, 1 moving column/cycle at N=512) with fp32 PSUM accumulation:
  phase 1: H^T[j,t]   = relu( sum_c A^T[c,j] * X^T[c,t] )  (j=4096, t=1024, c=1024)
  phase 2: Y^T[c,t]   = sum_j B[j,c] * H^T[j,t]
1024 matmuls of [128x128]@[128x512] at ~216 ns warm => ~221.7 us of PE stream.

v2 changes vs the 242 us baseline (trace-driven):
 - input head is packed into 5 "slab" dram tensors ([xt0|at0], [xt1|at1],
   [xt2|xt3], [xt4|xt5], [xt6|xt7]) DMA'd sequentially on the gpsimd SWDGE
   queue: 512KB descriptors reach much higher per-queue bandwidth than the
   baseline's 256KB tiles, and sequential issue gives staircase completion,
   so the first real matmul can start at ~9.5 us instead of ~14.4 us.
 - warmup matmul count cut from 18 to a handful: just enough to cover the
   slab-0 DMA wait and pre-warm the HAM clock gate; the baseline's warmup
   kept the PE busy ~6 us past the point the real inputs had landed.
 - A-tile stream (jt>=2) prefetched 4 tiles deep on the two HWDGE queues
   (sync/scalar alternating).
 - phase-2 final accumulation group (th=1, c8=7) split into 2x N=256 groups;
   the first half's copy+DMA hides under the second half's matmuls and the
   last half is emitted fp16 (64KB -> 32KB) to shrink the exposed tail.
"""

import os
import sys

import numpy as np


def _ensure_concourse():
    try:
        import concourse  # noqa: F401
    except ImportError:
        for p in ("/opt/trn_rl_repo", "/root/.axon_site/_ro/trn_rl_repo"):
            if os.path.isdir(p) and p not in sys.path:
                sys.path.insert(0, p)
        import concourse  # noqa: F401


N_EMBD = 1024
T_SEQ = 1024
ITER = 4096
FULL = 12288
R_NOVELTY = 0.7
NCORES = 8
JT = ITER // 128   # 32 j-tiles
CT = N_EMBD // 128  # 8 c-tiles
HEAD_JT = 2        # j-tiles computed ct-outer while the x stream lands
N_WARM = 5         # warmup matmuls bridging the slab-0 DMA wait

# slab layout: hx[s] is [128, 2048] fp16; map ct -> (slab, column offset)
_XT_SLOT = {0: (0, 0), 1: (1, 0), 2: (2, 0), 3: (2, 1024),
            4: (3, 0), 5: (3, 1024), 6: (4, 0), 7: (4, 1024)}
_AT_SLOT = {0: (0, 1024), 1: (1, 1024)}   # at0 / at1 ride in slabs 0/1

_STATE: dict = {}


# ---------------------------------------------------------------- device side

def _build_nc():
    _ensure_concourse()
    import concourse.tile as tile
    from concourse import bacc, mybir
    from concourse.bass import ts

    f32 = mybir.dt.float32
    f16 = mybir.dt.float16
    relu = mybir.ActivationFunctionType.Relu

    nc = bacc.Bacc("TRN2", target_bir_lowering=False, debug=False,
                   num_devices=NCORES)
    hxa = nc.dram_tensor("hx", [5, 128, 2048], f16, kind="ExternalInput").ap()
    aa = nc.dram_tensor("at", [JT - HEAD_JT, 128, N_EMBD], f16,
                        kind="ExternalInput").ap()
    ba = nc.dram_tensor("bt", [8, 128, 4 * N_EMBD], f16,
                        kind="ExternalInput").ap()
    # output is Y^T tiles: [t-half, c-tile, 128 c, 512 t]; the final half
    # group (th=1, c8=7, t 256:512) is emitted separately in fp16.
    ya = nc.dram_tensor("y", [2, 8, 128, 512], f32, kind="ExternalOutput").ap()
    y2a = nc.dram_tensor("y2", [128, 256], f16, kind="ExternalOutput").ap()

    with tile.TileContext(nc) as tc:
        with (
            tc.tile_pool(name="hx", bufs=5) as hx_pool,
            tc.tile_pool(name="wm", bufs=1) as wm_pool,
            tc.tile_pool(name="ht", bufs=JT) as ht_pool,
            tc.tile_pool(name="at", bufs=6) as at_pool,
            tc.tile_pool(name="bt", bufs=8) as bt_pool,
            tc.tile_pool(name="yo", bufs=4) as yo_pool,
            tc.tile_pool(name="yz", bufs=2) as yz_pool,
            tc.tile_pool(name="ps", bufs=8, space="PSUM") as ps_pool,
        ):
            # input slabs: 512KB sequential DMAs on the gpsimd SWDGE queue.
            hx_t = []
            for s in range(5):
                t = hx_pool.tile([128, 2048], f16, tag="hx", name=f"hx{s}")
                nc.gpsimd.dma_start(out=t[:], in_=hxa[s])
                hx_t.append(t)

            def xt_ap(ct):
                s, off = _XT_SLOT[ct]
                return hx_t[s][:, off:off + 1024]

            def at_head_ap(jt):
                s, off = _AT_SLOT[jt]
                return hx_t[s][:, off:off + 1024]

            # A-tile stream for jt>=2 on the two HWDGE queues, prefetched.
            att = {}

            def at_dma(jt):
                t = at_pool.tile([128, N_EMBD], f16, tag="at", name=f"a{jt}")
                eng = nc.sync if jt % 2 == 0 else nc.scalar
                eng.dma_start(out=t[:], in_=aa[jt - HEAD_JT])
                att[jt] = t

            for jt in range(HEAD_JT, HEAD_JT + 4):
                at_dma(jt)

            # PE warmup on a zeroed tile: pre-warms the HAM clock gate and
            # covers the slab-0 DMA wait without delaying the real stream.
            wt = wm_pool.tile([128, 512], f16, tag="warm", name="warm")
            nc.vector.memset(wt[:], 0)
            wps = ps_pool.tile([128, 512], f32, tag="ps", name="wps")
            for i in range(N_WARM):
                nc.tensor.matmul(wps[:], lhsT=wt[:, ts(0, 128)], rhs=wt[:],
                                 start=True, stop=True)

            # phase 1 head: j-tiles 0..HEAD_JT-1 run ct-outer so matmuls
            # start as soon as slab s lands (ct arrival order = slab order).
            head_ps = [[ps_pool.tile([128, 512], f32, tag="ps",
                                     name=f"hps{jt}_{i}") for i in range(2)]
                       for jt in range(HEAD_JT)]
            hts = []
            for ct in range(CT):
                for jt in range(HEAD_JT):
                    lhs = at_head_ap(jt)[:, ts(ct, 128)]
                    xap = xt_ap(ct)
                    for th in range(2):
                        nc.tensor.matmul(
                            head_ps[jt][th][:], lhsT=lhs,
                            rhs=xap[:, ts(th, 512)],
                            start=(ct == 0), stop=(ct == CT - 1))
            for jt in range(HEAD_JT):
                htt = ht_pool.tile([128, T_SEQ], f16, tag="ht", name=f"h{jt}")
                for th in range(2):
                    nc.scalar.activation(htt[:, ts(th, 512)],
                                         head_ps[jt][th][:], relu)
                hts.append(htt)

            # phase 1 main loop: stationary A-tile per jt, 16 matmuls.
            for jt in range(HEAD_JT, JT):
                if jt + 4 < JT:
                    at_dma(jt + 4)
                a = att[jt]
                ps = [ps_pool.tile([128, 512], f32, tag="ps",
                                   name=f"ps{jt}_{i}") for i in range(2)]
                for ct in range(CT):
                    lhs = a[:, ts(ct, 128)]
                    for th in range(2):
                        nc.tensor.matmul(
                            ps[th][:], lhsT=lhs,
                            rhs=xt_ap(ct)[:, ts(th, 512)],
                            start=(ct == 0), stop=(ct == CT - 1))
                htt = ht_pool.tile([128, T_SEQ], f16, tag="ht", name=f"h{jt}")
                for th in range(2):
                    nc.scalar.activation(htt[:, ts(th, 512)], ps[th][:], relu)
                hts.append(htt)

            # B tiles: 1MB DMAs on gpsimd behind the slabs; resident fp16.
            bts = []
            for g in range(8):
                btt = bt_pool.tile([128, 4 * N_EMBD], f16, tag="bt",
                                   name=f"b{g}")
                nc.gpsimd.dma_start(out=btt[:], in_=ba[g])
                bts.append(btt)

            def bt_ap(jt, c8):
                return bts[jt // 4][:, (jt % 4) * N_EMBD + 128 * c8:
                                    (jt % 4) * N_EMBD + 128 * (c8 + 1)]

            # phase 2: Y^T[c,t] accumulated over j. c8-outer/jt-inner so all
            # copies/output DMAs except the last hide under the MM stream.
            for th in range(2):
                for c8 in range(8):
                    if th == 1 and c8 == 7:
                        break
                    pst = ps_pool.tile([128, 512], f32, tag="ps",
                                       name=f"yps{th}_{c8}")
                    for jt in range(JT):
                        nc.tensor.matmul(
                            pst[:], lhsT=bt_ap(jt, c8),
                            rhs=hts[jt][:, ts(th, 512)],
                            start=(jt == 0), stop=(jt == JT - 1))
                    yo = yo_pool.tile([128, 512], f32, tag="yo",
                                      name=f"y{th}_{c8}")
                    if c8 % 2 == 0:
                        nc.vector.tensor_copy(yo[:], pst[:])
                        nc.sync.dma_start(out=ya[th, c8], in_=yo[:])
                    else:
                        nc.scalar.copy(yo[:], pst[:])
                        nc.scalar.dma_start(out=ya[th, c8], in_=yo[:])

            # final group (th=1, c8=7) split into 2x N=256 so the exposed
            # tail is one small fp16 transfer instead of a 256KB fp32 one.
            psA = ps_pool.tile([128, 256], f32, tag="ps", name="ypsA")
            for jt in range(JT):
                nc.tensor.matmul(psA[:], lhsT=bt_ap(jt, 7),
                                 rhs=hts[jt][:, 512:768],
                                 start=(jt == 0), stop=(jt == JT - 1))
            yoA = yz_pool.tile([128, 256], f32, tag="yz", name="yA")
            nc.vector.tensor_copy(yoA[:], psA[:])
            nc.sync.dma_start(out=ya[1, 7, :, 0:256], in_=yoA[:])

            psB = ps_pool.tile([128, 256], f32, tag="ps", name="ypsB")
            for jt in range(JT):
                nc.tensor.matmul(psB[:], lhsT=bt_ap(jt, 7),
                                 rhs=hts[jt][:, 768:1024],
                                 start=(jt == 0), stop=(jt == JT - 1))
            yoB = yz_pool.tile([128, 256], f16, tag="yz", name="yB")
            nc.scalar.copy(yoB[:], psB[:])
            nc.scalar.dma_start(out=y2a[:], in_=yoB[:])

    nc.compile()
    return nc


class _Runner:
    """Persistent jitted SPMD dispatcher (mirrors bass2jax.run_bass_via_pjrt's
    multi-core branch, but reuses one jax.jit across calls)."""

    def __init__(self, nc):
        _ensure_concourse()
        import jax
        import concourse.mybir as mybir
        from concourse import bass2jax
        from jax.experimental.shard_map import shard_map
        from jax.sharding import Mesh, PartitionSpec

        bass2jax.install_neuronx_cc_hook()
        self.nc = nc
        partition_name = (nc.partition_id_tensor.name
                          if nc.partition_id_tensor else None)
        in_names, out_names, out_avals, zero_shapes = [], [], [], []
        for alloc in nc.m.functions[0].allocations:
            if not isinstance(alloc, mybir.MemoryLocationSet):
                continue
            name = alloc.memorylocations[0].name
            if alloc.kind == "ExternalInput":
                if name != partition_name:
                    in_names.append(name)
            elif alloc.kind == "ExternalOutput":
                shape = tuple(alloc.tensor_shape)
                dtype = mybir.dt.np(alloc.dtype)
                out_names.append(name)
                out_avals.append(jax.core.ShapedArray(shape, dtype))
                zero_shapes.append((shape, dtype))
        self.in_names = list(in_names)
        self.out_names = out_names
        self.out_avals = out_avals
        self.zero_shapes = zero_shapes
        n_params = len(in_names)
        all_in_names = in_names + out_names
        if partition_name is not None:
            all_in_names.append(partition_name)

        def _body(*args):
            operands = list(args)
            if partition_name is not None:
                operands.append(bass2jax.partition_id_tensor())
            outs = bass2jax._bass_exec_p.bind(
                *operands,
                out_avals=tuple(out_avals),
                in_names=tuple(all_in_names),
                out_names=tuple(out_names),
                lowering_input_output_aliases=(),
                sim_require_finite=True,
                sim_require_nnan=True,
                nc=nc,
            )
            return tuple(outs)

        devices = jax.devices()[:NCORES]
        assert len(devices) == NCORES
        self.mesh = Mesh(np.asarray(devices), ("core",))
        n_outs = len(out_names)
        in_specs = (PartitionSpec("core"),) * (n_params + n_outs)
        out_specs = (PartitionSpec("core"),) * n_outs
        self.donate = tuple(range(n_params, n_params + n_outs))
        self.fn = jax.jit(
            shard_map(_body, mesh=self.mesh, in_specs=in_specs,
                      out_specs=out_specs, check_rep=False),
            donate_argnums=self.donate, keep_unused=True)

    def concat_inputs(self, in_maps):
        return [np.concatenate([np.asarray(m[n]) for m in in_maps], axis=0)
                for n in self.in_names]

    def zero_outs(self):
        return [np.zeros((NCORES * s[0], *s[1:]), d)
                for (s, d) in self.zero_shapes]

    def __call__(self, in_maps):
        concat_in = self.concat_inputs(in_maps)
        out_arrs = self.fn(*concat_in, *self.zero_outs())
        return [
            {n: np.asarray(out_arrs[i]).reshape(NCORES, *self.out_avals[i].shape)[c]
             for i, n in enumerate(self.out_names)}
            for c in range(NCORES)
        ]


def _get_runner():
    if "runner" not in _STATE:
        nc = _build_nc()
        _STATE["nc"] = nc
        _STATE["runner"] = _Runner(nc)
    return _STATE["runner"]


# ------------------------------------------------------------------ host side

def _tile_A(A):
    """(4096, 1024) row-gathered Wm -> fp16 'at' tiles [jt, ci, ct*128+jj]."""
    return np.ascontiguousarray(
        A.reshape(JT, 128, CT, 128).transpose(0, 3, 2, 1)).reshape(
            JT, 128, N_EMBD).astype(np.float16)


def _tile_B(Bm):
    """(4096, 1024) row-gathered Wp^T -> fp16 'bt' layout [8, jj, 4jt*c]."""
    t = np.ascontiguousarray(Bm).reshape(8, 4, 128, N_EMBD).astype(np.float16)
    return np.ascontiguousarray(t.transpose(0, 2, 1, 3)).reshape(
        8, 128, 4 * N_EMBD)


def _tile_X(xc):
    """(B, 1024 t, 1024 c) -> per-core fp16 xt tiles [B, ct, ci, t]."""
    return np.ascontiguousarray(xc.transpose(0, 2, 1)).reshape(
        xc.shape[0], CT, 128, T_SEQ).astype(np.float16)


def _pack_hx(xts_core, at_tiles):
    """Interleave xt tiles + at0/at1 into the 5 slab tensors [5,128,2048]."""
    hx = np.empty((5, 128, 2048), np.float16)
    for ct, (s, off) in _XT_SLOT.items():
        hx[s, :, off:off + 1024] = xts_core[ct]
    for jt, (s, off) in _AT_SLOT.items():
        hx[s, :, off:off + 1024] = at_tiles[jt]
    return hx


def _untile_Y(res):
    """{'y': [th,c8,ci,tt] f32, 'y2': [ci,256] f16} -> f32 (1024 t, 1024 c)."""
    y = np.ascontiguousarray(
        res["y"].transpose(0, 3, 1, 2)).reshape(T_SEQ, N_EMBD).astype(
            np.float32, copy=False)
    y[768:1024, 896:1024] = res["y2"].T.astype(np.float32)
    return y


def _device_forward(xc, hx_list, at_list, bt_list):
    """y[b] = relu(xc[b] @ A^T) @ B for 8 cores at once."""
    in_maps = []
    for b in range(NCORES):
        in_maps.append({"hx": hx_list[b], "at": at_list[b], "bt": bt_list[b]})
    try:
        results = _get_runner()(in_maps)
    except Exception:
        # fall back to the supported dispatch path (fresh jit per call)
        from concourse.bass_utils import run_bass_kernel_spmd
        if "nc" not in _STATE:
            _STATE["nc"] = _build_nc()
        results = run_bass_kernel_spmd(
            _STATE["nc"], in_maps, list(range(NCORES))).results
    return np.stack([_untile_Y(results[b]) for b in range(NCORES)])


def _topk_mask(ck, k):
    # matches jax.lax.top_k tie-breaking (first index wins) via stable argsort
    order = np.argsort(-ck, axis=1, kind="stable")[:, :k]
    mask = np.zeros_like(ck)
    np.put_along_axis(mask, order, 1.0, axis=1)
    return mask


def kernel(x, Wm, Wp, Wc1, Wc2):
    x = np.ascontiguousarray(np.asarray(x, dtype=np.float32))
    Wm = np.ascontiguousarray(np.asarray(Wm, dtype=np.float32))
    Wp = np.ascontiguousarray(np.asarray(Wp, dtype=np.float32))
    Wc1 = np.asarray(Wc1, dtype=np.float32)
    Wc2 = np.asarray(Wc2, dtype=np.float32)
    B = x.shape[0]
    assert B == NCORES and x.shape[1] == T_SEQ and x.shape[2] == N_EMBD

    WpT = None  # lazily built; only needed on non-base iterations
    base = np.zeros((B, FULL), np.float32)
    base[:, :ITER] = 1.0

    xc, active, history, done = x, base, base.copy(), False
    for _ in range(3):
        if done:
            break
        idxs = [np.flatnonzero(active[b]) for b in range(B)]
        is_base = all(ix.shape[0] == ITER and ix[0] == 0 and ix[-1] == ITER - 1
                      for ix in idxs) and all(
                          np.array_equal(ix, idxs[0]) for ix in idxs[1:])
        xts = _tile_X(xc)
        if is_base and np.array_equal(idxs[0], np.arange(ITER)):
            at = _tile_A(Wm[:ITER])
            bt = _tile_B(np.ascontiguousarray(Wp[:, :ITER].T))
            at_tail = np.ascontiguousarray(at[HEAD_JT:])
            hx_list = [_pack_hx(xts[b], at) for b in range(B)]
            at_list = [at_tail] * B
            bt_list = [bt] * B
        else:
            if WpT is None:
                WpT = np.ascontiguousarray(Wp.T)
            ats = [_tile_A(np.ascontiguousarray(Wm[ix])) for ix in idxs]
            hx_list = [_pack_hx(xts[b], ats[b]) for b in range(B)]
            at_list = [np.ascontiguousarray(a[HEAD_JT:]) for a in ats]
            bt_list = [_tile_B(WpT[ix]) for ix in idxs]

        y = _device_forward(xc, hx_list, at_list, bt_list)

        pooled = y.mean(axis=1)
        ck = np.maximum(pooled @ Wc1.T, 0.0) @ Wc2.T
        new_mask = _topk_mask(ck, ITER)
        combined = np.clip(history + new_mask, 0.0, 1.0)
        novelty = (combined - history).sum(axis=1).mean() / ITER
        xc, active, history = y, new_mask, combined
        done = bool(novelty < R_NOVELTY)

    return xc.astype(np.float32, copy=False)
